# revision 60
# baseline (speedup 1.0000x reference)
"""Neighbor-slice attention (nn_AttentionModule) on 8 TRN2 NeuronCores.

Per core: 2 of 16 slices + 1 halo slice each side, packed by the host.
~192us HW exec (baseline was 469us).  The performance story on TRN2:

- The PE HAM clock gate only reaches 2.4 GHz when the tensor engine runs
  dense FULL-ARRAY matmuls; q/k are zero-padded to a 128-deep contraction
  (64 real ci rows + 64 zeros) purely so the activity monitor un-throttles.
  Any multi-us PE bubble re-latches 1.2 GHz, so the whole kernel is built
  around a gap-free PE instruction stream.
- All matmuls fp16/bf16 (1 cycle/row).  f = k_chunk^T q runs fp16 (10-bit
  mantissa keeps softmax logits accurate); exp outputs / v / Wz are bf16.
- Per q-block (<=512 cols, one PSUM bank) BOTH neighbor sides run
  interleaved with a one-j skew:  f0_j, f1_j, y0_{j-1}, y1_{j-1} -- each
  exp gets ~5 matmuls of cover before its y-matmul consumes it.  The
  FIRST y-pair of each unit is deferred one slot (y0/y1 emitted together
  at j==2) so it never stalls on the previous unit's yps bank still
  draining through its ycop; the j==1 PE hole takes the deferred z
  matmuls (moved from j3 to j1, output evac j5 to j3).
- exp alternates strictly between ACT (true exp) and DVE (Schraudolph
  bf16 fast-exp) within every j-pair, so neither FIFO ever takes two
  exps of one pair; the steady-state attention stream is gap-free.
- The y-matmul lhsT carries a ones column at index 0, so yps partition 0
  accumulates the softmax denominator for free (and custom-DVE
  reciprocal_approx_fast can only read partition 0).
- Unit tails never block the strict-FIFO ACT/DVE queues: yps is copied
  whole to SBUF (ACT + DVE, one each) at unit end; recips flush at j5/j7
  and the 1/denom multiplies at j15/j16 of the NEXT unit (past the
  boundary-bunch drain window, inputs long ready); the z matmuls,
  residual identity-matmul and c2-bias output evac (ACT) flush at j3/j5
  TWO units later -- a two-generation software pipeline of the tails.
- Residual x enters via an identity matmul (fp16) into zps; c2=2(Wz bv+bz)
  is applied as an ACT bias on the final evacuation; bq/bk are applied on
  the projection evacuations (qt on ACT, kt on DVE, pipelined in thirds).
- Host supplies x in fp16 (no on-chip casts), plus small constant tensors
  (identity, vg ones/zeros padding) to keep slow gpsimd memsets off the
  critical path.
"""

import sys

for _p in ("/opt/trn_rl_repo",):
    if _p not in sys.path:
        sys.path.insert(0, _p)

import numpy as np

N_FULL, C, H, W = 16, 128, 48, 48
HW = H * W            # 2304
CI = C // 2           # 64
KC = HW // 128        # 18 k-chunks per slice
NCORES = 8
NLOC = N_FULL // NCORES  # 2 local slices per core

# q-blocks (start, width); width <= 512 (one PSUM bank)
QBS = [(0, 512), (512, 512), (1024, 512), (1536, 512), (2048, 256)]

# Schraudolph fast-exp on DVE for a subset of (n, side, qi, j) units to
# offload the ACT engine.
EXP_DVE_PRED = lambda n, side, qi, j: (j % 2 == 1)

# bf16 Schraudolph constants: bits16 = round(x*log2(e)*128 + B16)
_S16 = 184.66496736235803          # 2**7 / ln(2)
_B16 = 16256.0 - 4.75              # 127*2**7 with mid-sawtooth correction

_NC_CACHE = {}
LAST_RESULTS = None
TRACE = False


def _build_nc():
    import concourse.bass as bass
    import concourse.mybir as mybir
    import concourse.tile as tile
    from concourse import bacc

    f32 = mybir.dt.float32
    bf16 = mybir.dt.bfloat16
    f16 = mybir.dt.float16
    i16 = mybir.dt.int16
    FT = mybir.ActivationFunctionType

    nc = bacc.Bacc()

    xh_d = nc.declare_dram_parameter("xh", [4, C, HW], f16, isOutput=False)
    wqk_d = nc.declare_dram_parameter("wqk", [C, C], f16, isOutput=False)
    wv_d = nc.declare_dram_parameter("wv", [C, CI], f16, isOutput=False)
    wz_d = nc.declare_dram_parameter("wz", [CI, C], bf16, isOutput=False)
    bqk_d = nc.declare_dram_parameter("bqk", [C, 1], f32, isOutput=False)
    c2_d = nc.declare_dram_parameter("c2", [C, 1], f32, isOutput=False)
    ident_d = nc.declare_dram_parameter("ident", [C, C], f16, isOutput=False)
    vgpad_d = nc.declare_dram_parameter("vgpad", [C, KC, CI], bf16,
                                        isOutput=False)
    out_d = nc.declare_dram_parameter("out", [NLOC, C, HW], f32, isOutput=True)

    with tile.TileContext(nc) as tc:
        with tc.tile_pool(name="const", bufs=1) as cpool, \
             tc.tile_pool(name="xb", bufs=4) as xbpool, \
             tc.tile_pool(name="qt", bufs=2) as qtpool, \
             tc.tile_pool(name="kt", bufs=4) as ktpool, \
             tc.tile_pool(name="vg", bufs=4) as vgpool, \
             tc.tile_pool(name="at", bufs=12) as atpool, \
             tc.tile_pool(name="ysb", bufs=4) as ysbpool, \
             tc.tile_pool(name="ycop", bufs=6) as ycpool, \
             tc.tile_pool(name="rr", bufs=4) as rrpool, \
             tc.tile_pool(name="rb", bufs=4) as rbpool, \
             tc.tile_pool(name="osb", bufs=6) as opool:

            # ---- constants (host supplies f16/bf16 directly) ----
            wqk_t = cpool.tile([C, C], f16, tag="wqk")
            wv_t = cpool.tile([C, CI], f16, tag="wv")
            wz_t = cpool.tile([CI, C], bf16, tag="wz")
            bqk_t = cpool.tile([C, 1], f32, tag="bqk")
            c2_t = cpool.tile([C, 1], f32, tag="c2")
            id_t = cpool.tile([C, C], f16, tag="id")
            ones1_t = cpool.tile([1, CI], f32, tag="ones1")

            # ---- load features first: the projection critical path needs
            # wqk + xh[0] before anything else (each dma_start costs ~0.7us
            # of serial issue on the sync queue) ----
            nc.sync.dma_start(out=wqk_t, in_=wqk_d[:, :])
            xb_t = []
            for s in range(4):
                xb = xbpool.tile([C, HW], f16, tag="xb")
                xb_t.append(xb)
            nc.sync.dma_start(out=xb_t[0], in_=xh_d[0])
            nc.sync.dma_start(out=wv_t, in_=wv_d[:, :])
            nc.sync.dma_start(out=bqk_t, in_=bqk_d[:, :])
            for s in range(1, 4):
                nc.sync.dma_start(out=xb_t[s], in_=xh_d[s])
            nc.sync.dma_start(out=wz_t, in_=wz_d[:, :])
            nc.sync.dma_start(out=c2_t, in_=c2_d[:, :])
            nc.sync.dma_start(out=id_t, in_=ident_d[:, :])
            nc.gpsimd.memset(ones1_t, 1.0)


            # ---- projections (all f16); pq in thirds, double-buffered so
            # the PE never waits on the evacuations (qt on ACT, kt on DVE) --
            qt_t = [None, None]      # local slices only (x4 idx 1, 2)
            kt_t = [None] * 4
            vg_t = [None] * 4
            for s in range(4):
                kt = ktpool.tile([C, HW], f16, tag="kt", name=f"kt{s}")
                nc.gpsimd.memset(kt[CI:C, :], 0.0)
                kt_t[s] = kt
            for s in (1, 2):
                qt = qtpool.tile([C, HW], f16, tag="qt", name=f"qt{s}")
                nc.gpsimd.memset(qt[CI:C, :], 0.0)
                qt_t[s - 1] = qt
            T3 = HW // 3             # 768
            with tc.tile_pool(name="pp", bufs=2, space="PSUM") as pp, \
                 tc.tile_pool(name="pv", bufs=2, space="PSUM") as pv:
                for s in range(4):
                    kt = kt_t[s]
                    vg = vgpool.tile([C, KC, 2 * CI], bf16, tag="vg",
                                     name=f"vg{s}")
                    nc.sync.dma_start(out=vg[:, :, 0:CI],
                                      in_=vgpad_d[:, :, :])
                    vg_t[s] = vg
                    KH = KC // 2
                    pvt_h = [None, None]

                    def v_chunk(j, s=s, vg=vg, pvt_h=pvt_h):
                        h, jj = divmod(j, KH)
                        if jj == 0:
                            pvt_h[h] = pv.tile([C, KH * CI], f32, tag="pv",
                                               name=f"pvt{h}")
                        nc.tensor.matmul(
                            pvt_h[h][:, CI * jj:CI * (jj + 1)],
                            lhsT=xb_t[s][:, 128 * j:128 * (j + 1)],
                            rhs=wv_t, start=True, stop=True)
                        if jj == KH - 1:
                            dst = vg[:, KH * h:KH * (h + 1), CI:2 * CI]
                            rsrc = pvt_h[h].rearrange("p (j d) -> p j d",
                                                      d=CI)
                            if h == 0:
                                nc.vector.tensor_copy(dst, rsrc)
                            else:
                                nc.scalar.activation(dst, rsrc, FT.Copy)

                    vidx = 0
                    for t in range(3):
                        t0 = T3 * t
                        pq = pp.tile([C, T3], f32, tag="pp")
                        for (b0, bw) in ((0, 512), (512, 256)):
                            nc.tensor.matmul(pq[:, b0:b0 + bw], lhsT=wqk_t,
                                             rhs=xb_t[s][:, t0 + b0:t0 + b0 + bw],
                                             start=True, stop=True)
                        for _ in range(6):
                            v_chunk(vidx)
                            vidx += 1
                        if s in (1, 2):
                            nc.scalar.activation(qt_t[s - 1][0:CI, t0:t0 + T3],
                                                 pq[0:CI, :], FT.Identity,
                                                 bias=bqk_t[0:CI, :])
                        nc.vector.tensor_scalar_add(kt[0:CI, t0:t0 + T3],
                                                    pq[CI:C, :],
                                                    bqk_t[CI:C, :])

            # ---- attention ----
            # Both sides (before/after) of each q-block run concurrently with
            # a one-j skew between f and y matmuls; per j the PE queue is
            #   f0_j, f1_j, y0_{j-1}, y1_{j-1}
            # which gives each exp 5 matmuls (~1.1us warm) of cover before its
            # y needs it.  The unit tail is engineered to never block the
            # strict-FIFO ACT/DVE queues (head-of-line blocking there starves
            # the PE and re-latches the HAM clock gate to 1.2 GHz):
            #   - at unit end: yps row-copies to SBUF (one on ACT, one on
            #     DVE), recips (DVE, data already available), then broadcast
            #     and the 1/denom multiply run entirely on idle GPSIMD
            #   - z matmuls flush into the NEXT unit's PE queue at j==4,
            #     output evac (DVE) + DMA at j==6
            with tc.tile_pool(name="pf", bufs=5, space="PSUM") as pf, \
                 tc.tile_pool(name="py", bufs=2, space="PSUM") as py, \
                 tc.tile_pool(name="pz", bufs=1, space="PSUM") as pz:
                pendA = {}
                pendB = {}
                pendB_next = {}
                eidx = 0
                for n in range(NLOC):
                    for qi, (q0, w) in enumerate(QBS):
                        zps = pz.tile([C, w], f32, tag="zps")
                        yps = [py.tile([2 * CI, w], f32, tag="yps",
                                       name=f"yps{s}")
                               for s in range(2)]
                        at_prev = [None, None]
                        at_hist = {}
                        for j in range(KC):
                            at_cur = [None, None]
                            for side in range(2):
                                kv = n + 2 * side
                                ft = pf.tile([C, w], f32, tag="ft")
                                nc.tensor.matmul(
                                    ft,
                                    lhsT=kt_t[kv][:, 128 * j:128 * (j + 1)],
                                    rhs=qt_t[n][:, q0:q0 + w],
                                    start=True, stop=True)
                                at = atpool.tile([C, w], bf16, tag="at")
                                pj = eidx // 2
                                use_dve = (side + pj) % 2 == 1
                                if j == KC - 1:
                                    use_dve = side == 1
                                if use_dve:
                                    nc.vector.tensor_scalar(
                                        at.bitcast(i16), ft, _S16, _B16,
                                        op0=mybir.AluOpType.mult,
                                        op1=mybir.AluOpType.add)
                                else:
                                    nc.scalar.activation(at, ft, FT.Exp)
                                eidx += 1
                                at_cur[side] = at
                            for sched in (pendA, pendB):
                                if j in sched:
                                    for fn in sched.pop(j):
                                        fn()
                            # first y-pair deferred one slot (emitted with
                            # y(1) at j==2): the previous unit's yps bank
                            # is freed by its ycop ~1.3us into this unit,
                            # so a j==1 first-write stalls ~0.4us.  The
                            # j==1 PE hole takes the deferred z matmuls.
                            if j >= 1:
                                at_hist[j - 1] = at_prev
                            if j >= 3:
                                # two-slot y skew: every exp gets a full
                                # extra slot of cover (kills the ~370ns
                                # y-waits-exp gap seen once per unit)
                                for jj in sorted(at_hist):
                                    atp = at_hist.pop(jj)
                                    for side in range(2):
                                        kv = n + 2 * side
                                        nc.tensor.matmul(
                                            yps[side],
                                            lhsT=vg_t[kv][:, jj, :],
                                            rhs=atp[side],
                                            start=(jj == 0), stop=False)
                            at_prev = at_cur
                        for jj in sorted(at_hist):
                            atp = at_hist.pop(jj)
                            for side in range(2):
                                kv = n + 2 * side
                                nc.tensor.matmul(yps[side],
                                                 lhsT=vg_t[kv][:, jj, :],
                                                 rhs=atp[side],
                                                 start=(jj == 0), stop=False)
                        for side in range(2):
                            kv = n + 2 * side
                            nc.tensor.matmul(yps[side],
                                             lhsT=vg_t[kv][:, KC - 1, :],
                                             rhs=at_prev[side],
                                             start=False, stop=True)
                        # --- unit tail: full-width yps copies to SBUF (the
                        # only yps readers, so the PSUM banks free instantly);
                        # recip/broadcast/multiply all read the SBUF copies
                        # and flush as singletons into the next unit's queues
                        ycop = []
                        for side in range(2):
                            yc = ycpool.tile([C, w], f32, tag="ycop",
                                             name=f"ycop{side}")
                            if side == 0:
                                nc.scalar.activation(yc, yps[0], FT.Copy)
                            else:
                                nc.vector.tensor_copy(yc, yps[1])
                            ycop.append(yc)
                        ysbs = []
                        tail_ops = []
                        for side in range(2):
                            ysb = ysbpool.tile([CI, w], bf16, tag="ysb",
                                               name=f"ysb{side}")
                            ysbs.append(ysb)

                            def mk_recip(side=side, w=w):
                                def do_recip():
                                    rrow = rrpool.tile([1, w], f32, tag="rr",
                                                       name="rrow")
                                    nc.vector.reciprocal_approx_fast(
                                        rrow, ycop[side][0:1, :])
                                    rb64 = rbpool.tile([C, w], f32, tag="rb",
                                                       name="rb64")
                                    nc.gpsimd.partition_broadcast(rb64, rrow)
                                    return rb64
                                return do_recip

                            def mk_mul(side=side, ysb=ysb, ycop=ycop,
                                       zps=zps):
                                def do_mul(rb64):
                                    nc.vector.tensor_mul(
                                        ysb, ycop[side][CI:2 * CI, :],
                                        rb64[CI:2 * CI, :])
                                return do_mul

                            tail_ops.append((mk_recip(), mk_mul(side=side,
                                                                ysb=ysb)))

                        def make_z(zps=zps, ysbs=ysbs, n=n, q0=q0, w=w):
                            def flush_z():
                                for side in range(2):
                                    nc.tensor.matmul(zps, lhsT=wz_t,
                                                     rhs=ysbs[side],
                                                     start=(side == 0),
                                                     stop=False)
                                nc.tensor.matmul(zps, lhsT=id_t,
                                                 rhs=xb_t[n + 1][:, q0:q0 + w],
                                                 start=False, stop=True)
                            return flush_z

                        def make_out(zps=zps, n=n, q0=q0, w=w):
                            def flush_out():
                                osb = opool.tile([C, w], f32, tag="osb",
                                                 name="osb")
                                nc.scalar.activation(osb, zps, FT.Identity,
                                                     bias=c2_t)
                                nc.sync.dma_start(out=out_d[n][:, q0:q0 + w],
                                                  in_=osb)
                            return flush_out

                        rb_box = [None, None]

                        def mk_recip_fn(i, side):
                            def fn():
                                rb_box[side] = tail_ops[side][0]()
                            return fn

                        def mk_mul_fn(i, side):
                            def fn():
                                tail_ops[side][1](rb_box[side])
                            return fn

                        pendA = {
                            5: [mk_recip_fn(0, 0)],
                            7: [mk_recip_fn(0, 1)],
                            15: [mk_mul_fn(0, 0)],
                            16: [mk_mul_fn(0, 1)],
                        }
                        pendB = pendB_next
                        pendB_next = {
                            1: [make_z()],
                            3: [make_out()],
                        }
                for sched in (pendA, pendB, pendB_next):
                    for js in sorted(sched):
                        for fn in sched[js]:
                            fn()
                pendA = {}
                pendB = {}
                pendB_next = {}

    nc.compile()
    return nc


def _get_nc():
    if "nc" not in _NC_CACHE:
        _NC_CACHE["nc"] = _build_nc()
    return _NC_CACHE["nc"]


def _host_inputs(features, Wq, bq, Wk, bk, Wv, bv, Wz, bz):
    X = np.ascontiguousarray(np.asarray(features, np.float32).reshape(N_FULL, C, HW))
    wqk = np.ascontiguousarray(
        np.concatenate([Wq.T, Wk.T], axis=1).astype(np.float16))
    wv = np.ascontiguousarray(np.asarray(Wv).T.astype(np.float16))
    bqk = np.concatenate([bq, bk]).astype(np.float32).reshape(C, 1)
    c2 = (2.0 * (np.asarray(Wz) @ np.asarray(bv) + np.asarray(bz))).astype(
        np.float32).reshape(C, 1)
    ident = np.eye(C, dtype=np.float16)
    import ml_dtypes
    wz = np.ascontiguousarray(np.asarray(Wz).T.astype(ml_dtypes.bfloat16))
    vgpad = np.zeros((C, KC, CI), ml_dtypes.bfloat16)
    vgpad[:, :, 0] = 1.0
    in_maps = []
    for i in range(NCORES):
        idx = [max(2 * i - 1, 0), 2 * i, 2 * i + 1, min(2 * i + 2, N_FULL - 1)]
        in_maps.append({
            "xh": np.ascontiguousarray(X[idx].astype(np.float16)),
            "wqk": wqk, "wv": wv, "wz": wz, "bqk": bqk, "c2": c2,
            "ident": ident, "vgpad": vgpad,
        })
    return in_maps


def kernel(features, Wq, bq, Wk, bk, Wv, bv, Wz, bz):
    global LAST_RESULTS
    from concourse.bass_utils import run_bass_kernel_spmd

    nc = _get_nc()
    in_maps = _host_inputs(features, Wq, bq, Wk, bk, Wv, bv, Wz, bz)
    res = run_bass_kernel_spmd(nc, in_maps, core_ids=list(range(NCORES)),
                               trace=TRACE)
    LAST_RESULTS = res
    out = np.empty((N_FULL, C, H, W), np.float32)
    for i in range(NCORES):
        out[2 * i:2 * i + 2] = res.results[i]["out"].reshape(NLOC, C, H, W)
    return out

